# revision 51
# baseline (speedup 1.0000x reference)
"""Trainium2 Bass kernel for nn_CoreBlock (circulant attention + 2-layer FFN).

Contract: kernel(**inputs) takes FULL unsharded inputs (as produced by
setup_inputs) and returns the FULL [16, 1024, 768] f32 output.

Strategy: pure data-parallel over batch - 8 NeuronCores x 2 batches each.
All weights replicated. Per core (restructured for engine balance):

  phase A (per 4-chunk group): batched input DMA, bn_stats (DVE),
     rstd = recip(ACT Sqrt(var+eps)) (exact, 2 tiny ops), LayerNorm as a
     single ACT Copy(scale=rstd, bias=-mu*rstd), ONE batched XBAR
     DMA-transpose of the 4-chunk u tile, then v-projection matmuls.
     No PE transposes, no DVE normalize pass.
  phase B (per batch): circulant matmul with the 8-tile Toeplitz bank,
     free dim = jc-run * HS (<=256). Residual-added into X.
  phase C (per batch): 2x [Dense -> LayerNorm -> swish]. Row-sums of the
     dense output come free from a colsum column appended to Wf (PSUM col
     768); ssq via one DVE accum op; rstd via bit-hack + 1 Newton step
     (batched per group, all tiny DVE ops); Silu with scale/bias fused.
     Each 4-chunk group uses ONE batched XBAR DMA-transpose.
  tail (per batch): log_cosh(w) = |w| + log1p(exp(-2|w|)) - ln2 with the
     Exp/Ln table fence; outputs DMA'd per 4-chunk block on the gpsimd
     software queue.

Emission order A, B0, C0, B1, tail0, C1, tail1 lets batch-0's scalar/
vector tail run under batch-1's PE phases, so the PE never waits long
and the kernel ends shortly after the last matmul.

Matmul operands are bf16 (full-rate PE, fp32 PSUM accumulation); stats
and elementwise math fp32.
"""

import math
import numpy as np
import ml_dtypes

import concourse.bass as bass
import concourse.tile as tile
from concourse import bacc, mybir
from concourse.bass_utils import run_bass_kernel_spmd

BF16 = ml_dtypes.bfloat16

B, N, D = 16, 1024, 768
H, HS, L = 12, 64, 2
EPS = 1e-6
NCORES = 8
BPC = B // NCORES          # batches per core
NJ = N // 128              # token chunks per batch (8)
NT = BPC * NJ              # token chunks per core (16)
DC = D // 128              # feature chunks (6)
AB = 4                     # chunks per group
FW = 776                   # wf free width: 768 outputs + colsum + pad

F32 = mybir.dt.float32
I32 = mybir.dt.int32
BF = mybir.dt.bfloat16
Alu = mybir.AluOpType
Act = mybir.ActivationFunctionType

LN2 = math.log(2.0)
# fp32 whose bit pattern is 0x5f3759df (fast-rsqrt magic constant)
MAGIC_F = float(np.int32(0x5F3759DF).view(np.float32))

TRACE = False              # test harness sets this for profiling runs
TRACE_KW = {}
DEBUG = False              # adds intermediate-dump outputs (debugging only)

_cache = {}


class _Bacc(bacc.Bacc):
    """Bacc whose activation-table chooser sees Exp and Ln only in the
    combined natural_log_exp_and_others set, so the log_cosh tail needs one
    table load instead of alternating exp/ln loads."""

    def insert_act_table_loads(self):
        has_activation = any(
            isinstance(i, mybir.InstActivation)
            for b in self.main_func.blocks
            for i in b.instructions
        )
        if not has_activation:
            return
        from concourse.hw_specs import get_activation_tables
        Fn = mybir.ActivationFunctionType
        tables = []
        for name, fns in get_activation_tables(self.m.arch).items():
            if name != "natural_log_exp_and_others":
                fns = fns - {Fn.Exp, Fn.Ln}
            tables.append((name, fns))
        import concourse._compat as _compat  # noqa: F401
        from concourse.bacc import _bass_rust
        _bass_rust.insert_act_table_loads(self, tables)


def _build(cv_nonzero, bf_nonzero, lnf_uniform):
    nc = _Bacc("TRN2", target_bir_lowering=False, debug=False)

    # weights arrive pre-arranged as [partition, ...contiguous] so each
    # DMA is one large descriptor per partition
    xs = nc.dram_tensor("xs", (BPC, N, D), F32, kind="ExternalInput").ap()
    wv = nc.dram_tensor("wv", (128, DC, D), BF, kind="ExternalInput").ap()
    wf = nc.dram_tensor("wf", (128, L, DC, FW), BF, kind="ExternalInput").ap()
    tb_d = nc.dram_tensor("tbank", (128, H, NJ, 128), BF, kind="ExternalInput").ap()
    cv_d = nc.dram_tensor("cv", (D,), F32, kind="ExternalInput").ap()
    bf_d = nc.dram_tensor("bfb", (L, D), F32, kind="ExternalInput").ap()
    lnfs_d = nc.dram_tensor("lnfs", (L, D), F32, kind="ExternalInput").ap()
    lnfb_d = nc.dram_tensor("lnfb", (L, D), F32, kind="ExternalInput").ap()
    out_d = nc.dram_tensor("out", (BPC, N, D), F32, kind="ExternalOutput").ap()

    dbg = None
    if DEBUG:
        dbg = {
            "dbg_u": nc.dram_tensor("dbg_u", (128, AB, D), BF,
                                    kind="ExternalOutput").ap(),
            "dbg_v": nc.dram_tensor("dbg_v", (128, H, NJ, BPC, HS), BF,
                                    kind="ExternalOutput").ap(),
            "dbg_x1": nc.dram_tensor("dbg_x1", (128, BPC, NJ, D), F32,
                                     kind="ExternalOutput").ap(),
            "dbg_y2": nc.dram_tensor("dbg_y2", (128, AB, D), BF,
                                     kind="ExternalOutput").ap(),
            "dbg_z2": nc.dram_tensor("dbg_z2", (128, NJ, D), BF,
                                     kind="ExternalOutput").ap(),
        }

    with tile.TileContext(nc) as tc:
        _emit(nc, tc, xs, wv, wf, tb_d, cv_d, bf_d, lnfs_d, lnfb_d,
              out_d, cv_nonzero, bf_nonzero, lnf_uniform, dbg)
    nc.compile()
    return nc


def _newton1_rstd(nc, pool, dst, var_ap, magict, g):
    """dst[128, g] = 1/sqrt(var + EPS) via bit-hack seed + 1 Newton step
    (max rel err ~0.17%). All tiny DVE ops."""
    vv = pool.tile([128, 16], F32, tag="nvv", name="nvv")[:, 0:g]
    nc.vector.tensor_scalar(vv, var_ap, EPS, None, op0=Alu.add)
    y0 = pool.tile([128, 16], F32, tag="ny0", name="ny0")[:, 0:g]
    nc.vector.tensor_scalar(y0.bitcast(I32), vv.bitcast(I32), 1, None,
                            op0=Alu.logical_shift_right)
    nc.vector.tensor_tensor(y0.bitcast(I32), magict[:, 0:g].bitcast(I32),
                            y0.bitcast(I32), op=Alu.subtract)
    t1 = pool.tile([128, 16], F32, tag="nt1", name="nt1")[:, 0:g]
    nc.vector.tensor_tensor(t1, y0, y0, op=Alu.mult)
    nc.vector.tensor_tensor(t1, t1, vv, op=Alu.mult)
    nc.vector.tensor_scalar(t1, t1, -0.5, 1.5, op0=Alu.mult, op1=Alu.add)
    nc.vector.tensor_tensor(dst, y0, t1, op=Alu.mult)


def _emit(nc, tc, xs, wv, wf, tb_d, cv_d, bf_d, lnfs_d, lnfb_d,
          out_d, cv_nonzero, bf_nonzero, lnf_uniform, dbg=None):
    from contextlib import ExitStack
    ctx = ExitStack()
    with ctx:
        consts = ctx.enter_context(tc.tile_pool(name="consts", bufs=1))
        wp_tb = ctx.enter_context(tc.tile_pool(name="wp_tb", bufs=1))
        wp_wf = ctx.enter_context(tc.tile_pool(name="wp_wf", bufs=1))
        xpool = ctx.enter_context(tc.tile_pool(name="xpool", bufs=1))
        vpool = ctx.enter_context(tc.tile_pool(name="vpool", bufs=1))
        z2p = ctx.enter_context(tc.tile_pool(name="z2p", bufs=2))
        srcp = ctx.enter_context(tc.tile_pool(name="srcp", bufs=3))
        dtp = ctx.enter_context(tc.tile_pool(name="dtp", bufs=3))
        y1p = ctx.enter_context(tc.tile_pool(name="y1p", bufs=4))
        scrp = ctx.enter_context(tc.tile_pool(name="scrp", bufs=2))
        stat = ctx.enter_context(tc.tile_pool(name="stat", bufs=4))
        statp = ctx.enter_context(tc.tile_pool(name="statp", bufs=1))
        ps_mm = ctx.enter_context(tc.tile_pool(name="ps_mm", bufs=4, space="PSUM"))

        # ---- constants ----
        wv_s = consts.tile([128, DC, D], BF, tag="wv")
        # tb/wf DMAs are gated (below) so the 6.6MB of weights don't steal
        # HBM bandwidth from the input-x transfers; the gate tiles alias the
        # weight buffers and are read by a dummy op that depends on a late
        # input chunk's stats.
        tb_gate = wp_tb.tile([128, H, NJ, 128], BF, tag="tb", name="tb_gate")
        wf_gate = wp_wf.tile([128, L, DC, FW], BF, tag="wf", name="wf_gate")
        nc.vector.memset(tb_gate[:, 0, 0, 0:1], 0.0)
        nc.vector.memset(wf_gate[:, 0, 0, 0:1], 0.0)
        magict = consts.tile([128, 16], F32, tag="magic")
        nc.vector.memset(magict[:], MAGIC_F)
        onet = consts.tile([128, 1], F32, tag="one")
        nc.vector.memset(onet[:], 1.0)
        zerot = consts.tile([128, 1], F32, tag="zero")
        nc.vector.memset(zerot[:], 0.0)
        epst = consts.tile([128, 1], F32, tag="eps")
        nc.vector.memset(epst[:], EPS)

        # weight DMAs all go through the gpsimd software queue; tb/wf are
        # emitted mid-phase-A (below) so the input-x transfers get HBM
        # bandwidth first.
        nc.gpsimd.dma_start(wv_s[:], wv)

        cvt = None
        if cv_nonzero:
            cvt = consts.tile([128, D], F32, tag="cv")
            nc.gpsimd.dma_start(cvt[:], cv_d.to_broadcast((128, D)))
        bft = [None] * L
        lnfst = [None] * L
        lnfbt = [None] * L
        for l in range(L):
            if bf_nonzero[l]:
                bft[l] = consts.tile([128, D], F32, tag=f"bf{l}")
                nc.gpsimd.dma_start(bft[l][:], bf_d[l].to_broadcast((128, D)))
            if lnf_uniform[l] is None:
                lnfst[l] = consts.tile([128, D], F32, tag=f"lnfs{l}")
                nc.gpsimd.dma_start(lnfst[l][:], lnfs_d[l].to_broadcast((128, D)))
                lnfbt[l] = consts.tile([128, D], F32, tag=f"lnfb{l}")
                nc.gpsimd.dma_start(lnfbt[l][:], lnfb_d[l].to_broadcast((128, D)))

        # ---- resident tensors ----
        X = xpool.tile([128, BPC, NJ, D], F32, tag="X")         # x, then x1
        V = vpool.tile([128, H, NJ, BPC, HS], BF, tag="V")      # per-head values

        weights = {}
        # ================= phase A: LN + v-projection =================
        # LN's rstd commutes through the matmul: project (mu - x) @ Wv
        # (norm = one ACT Identity with bias = raw mean, nothing else on the
        # pre-matmul critical path), then scale V by -rstd in the post-matmul
        # copy. The reciprocal runs off-path on the DVE.
        mvA = statp.tile([128, NT, 2], F32, tag="mvA")
        nrsA = statp.tile([128, NT], F32, tag="nrsA")
        for g in range(NT // AB):
            b, jc0 = divmod(g * AB, NJ)
            # input DMAs per 2 chunks over three queues: few enough that the
            # DMA semaphore slots don't wrap onto the phase-A transposes
            # (WAR guards), spread for aggregate HBM bandwidth
            engs = (nc.sync, nc.scalar, nc.gpsimd)
            for h2 in range(AB // 2):
                jc = jc0 + h2 * 2
                eng = engs[(g * 2 + h2) % 3]
                eng.dma_start(
                    X[:, b, jc:jc + 2, :],
                    xs[b, jc * 128:(jc + 2) * 128, :].rearrange(
                        "(c p) d -> p c d", p=128))
            ug = srcp.tile([128, AB, D], BF, tag="src", name="ug")
            for ti in range(AB):
                t = g * AB + ti
                xt = X[:, b, jc0 + ti, :]
                st = stat.tile([128, 2, 6], F32, tag="bst")
                nc.vector.bn_stats(st[:, 0, :], xt[:, 0:512])
                nc.vector.bn_stats(st[:, 1, :], xt[:, 512:D])
                nc.vector.bn_aggr(mvA[:, t, :], st[:])
                # u~ = mu - x  (no rstd yet; fires right after this chunk's
                # aggr, independent of any other chunk)
                nc.scalar.activation(ug[:, ti, :], xt, Act.Identity,
                                     bias=mvA[:, t, 0:1], scale=-1.0)
            if g == 2:
                gd = stat.tile([128, 1], F32, tag="gd", name="gd_tb")
                nc.vector.scalar_tensor_tensor(
                    gd[:], mvA[:, g * AB + 3, 0:1], 1.0,
                    tb_gate[:, 0, 0, 0:1], op0=Alu.mult, op1=Alu.add)
                tb_s = wp_tb.tile([128, H, NJ, 128], BF, tag="tb")
                nc.gpsimd.dma_start(tb_s[:], tb_d)
                weights["tb"] = tb_s
            if g == 3:
                gd = stat.tile([128, 1], F32, tag="gd", name="gd_wf")
                nc.vector.scalar_tensor_tensor(
                    gd[:], mvA[:, g * AB + 3, 0:1], 1.0,
                    wf_gate[:, 0, 0, 0:1], op0=Alu.mult, op1=Alu.add)
                wf_s = wp_wf.tile([128, L, DC, FW], BF, tag="wf")
                nc.gpsimd.dma_start(wf_s[:], wf)
                weights["wf"] = wf_s
            gs = slice(g * AB, g * AB + AB)
            # -rstd = reciprocal(-sqrt(var+eps)); off the critical path
            stdt = stat.tile([128, 16], F32, tag="astd", name="astd")[:, 0:AB]
            nc.scalar.activation(stdt, mvA[:, gs, 1], Act.Sqrt, bias=epst[:])
            nstdt = stat.tile([128, 16], F32, tag="anstd", name="anstd")[:, 0:AB]
            nc.scalar.activation(nstdt, stdt, Act.Identity, scale=-1.0)
            nc.vector.reciprocal(nrsA[:, gs], nstdt)
            if dbg is not None and g == 0:
                nc.gpsimd.dma_start(dbg["dbg_u"], ug[:])
            for w in range(AB // 2):
              udt = dtp.tile([128, 2 * DC, 128], BF, tag="dt", name="udt")
              nc.sync.dma_start_transpose(
                  udt[:], ug[:, w * 2:w * 2 + 2, :].rearrange("p a d -> p (a d)"))
              for wi in range(2):
                ti = w * 2 + wi
                t = g * AB + ti
                jc = jc0 + ti
                pv = ps_mm.tile([128, 1024], F32, tag="mm")
                for c in range(DC):
                    nc.tensor.matmul(pv[:, 0:512], udt[:, wi * DC + c, :],
                                     wv_s[:, c, 0:512],
                                     start=(c == 0), stop=(c == DC - 1))
                    nc.tensor.matmul(pv[:, 512:D], udt[:, wi * DC + c, :],
                                     wv_s[:, c, 512:D],
                                     start=(c == 0), stop=(c == DC - 1))
                pv3 = pv[:, 0:D].rearrange("p (h k) -> p h k", h=H)
                if cv_nonzero:
                    cv3 = cvt[:].rearrange("p (h k) -> p h k", h=H)
                    nc.vector.scalar_tensor_tensor(V[:, :, jc, b, :], pv3,
                                                   nrsA[:, t:t + 1], cv3,
                                                   op0=Alu.mult, op1=Alu.add)
                else:
                    # V = -rstd * pv = rstd * (x - mu) @ Wv
                    nc.scalar.activation(V[:, :, jc, b, :], pv3, Act.Identity,
                                         scale=nrsA[:, t:t + 1])

        if dbg is not None:
            nc.gpsimd.dma_start(dbg["dbg_v"], V[:])

        # stats tiles for phase C
        ssqC = statp.tile([128, L, NT], F32, tag="ssqC")
        muC = statp.tile([128, L, NT], F32, tag="muC")
        rsC = statp.tile([128, L, NT], F32, tag="rsC")
        biasC = statp.tile([128, L, NT], F32, tag="biasC")
        fence = statp.tile([128, 4], F32, tag="fence")
        mvC = statp.tile([128, AB, 2], F32, tag="mvC")
        inv_d = 1.0 / D

        def phase_b(b):
            # y[ic] = sum_m T[m] @ V[(ic+m) % NJ]; residual into X.
            # two heads share one 2-bank PSUM tile (one accumulation bank
            # each) and get a single fused residual add, halving the DVE op
            # count and the PSUM WAR pressure.
            for h0 in range(0, H, 2):
                pc = ps_mm.tile([128, 2, NJ, HS], F32, tag="mm", name="pc")
                for hh in range(2):
                    h = h0 + hh
                    for m in range(NJ):
                        # one MM per run (free = ln*HS <= 512): a PSUM bank
                        # sees exactly one start=True MM
                        for ic0, jc0, ln in ((0, m, NJ - m), (NJ - m, 0, m)):
                            if ln == 0:
                                continue
                            nc.tensor.matmul(
                                pc[:, hh, ic0:ic0 + ln, :],
                                weights["tb"][:, h, m, :],
                                V[:, h, jc0:jc0 + ln, b, :],
                                start=(m == 0), stop=(m == NJ - 1),
                                skip_group_check=True)
                xap = X[:, b, :, h0 * HS:(h0 + 2) * HS]
                nc.vector.tensor_tensor(
                    xap, xap, pc[:].rearrange("p h j k -> p j h k"),
                    op=Alu.add)

        def c_epi(b, l, g, jc0, o, n, fast, y1g, pffs, y2g, Z2h,
                  var_src=None):
            """stats -> rstd -> Silu for chunk slice [o, o+n) of group g."""
            t0 = b * NJ + jc0 + o
            ts = slice(t0, t0 + n)
            if var_src is None:
                m2 = stat.tile([128, 16], F32, tag="m2", name="m2")[:, 0:n]
                nc.vector.tensor_scalar(m2, ssqC[:, l, ts], inv_d, None,
                                        op0=Alu.mult)
                var = stat.tile([128, 16], F32, tag="var", name="var")[:, 0:n]
                nc.vector.scalar_tensor_tensor(var, muC[:, l, ts], -1.0,
                                               muC[:, l, ts], op0=Alu.mult,
                                               op1=Alu.mult)
                nc.vector.tensor_tensor(var, m2, var, op=Alu.add)
            else:
                var = var_src
            _newton1_rstd(nc, stat, rsC[:, l, ts], var, magict, n)
            if fast:
                cs, cb = lnf_uniform[l]
                if cs != 1.0:
                    nc.vector.tensor_scalar(rsC[:, l, ts], rsC[:, l, ts],
                                            float(cs), None, op0=Alu.mult)
                nc.vector.scalar_tensor_tensor(biasC[:, l, ts], muC[:, l, ts],
                                               -1.0, rsC[:, l, ts],
                                               op0=Alu.mult, op1=Alu.mult)
                if cb != 0.0:
                    nc.vector.tensor_scalar(biasC[:, l, ts], biasC[:, l, ts],
                                            float(cb), None, op0=Alu.add)
                for k in range(n):
                    ti = o + k
                    t = t0 + k
                    ysrc = (pffs[ti][:, 0:D] if pffs[ti] is not None
                            else y1g[:, ti, 0:D])
                    dst = y2g[g][:, ti, :] if l == 0 else Z2h[:, jc0 + ti, :]
                    nc.scalar.activation(dst, ysrc, Act.Silu,
                                         bias=biasC[:, l, t:t + 1],
                                         scale=rsC[:, l, t:t + 1])
            else:
                for k in range(n):
                    ti = o + k
                    t = t0 + k
                    tmp = scrp.tile([128, D], BF, tag="scr", name="lnf_tmp")
                    nc.vector.tensor_scalar(tmp[:], y1g[:, ti, 0:D],
                                            muC[:, l, t:t + 1],
                                            rsC[:, l, t:t + 1],
                                            op0=Alu.subtract, op1=Alu.mult)
                    nc.vector.tensor_tensor(tmp[:], tmp[:], lnfst[l][:],
                                            op=Alu.mult)
                    dst = y2g[g][:, ti, :] if l == 0 else Z2h[:, jc0 + ti, :]
                    nc.vector.tensor_tensor(dst, tmp[:], lnfbt[l][:],
                                            op=Alu.add)
                    nc.scalar.activation(dst, dst, Act.Silu, bias=zerot[:])

        def phase_c(b, tail_cb=None):
            y2g = [None, None]
            xbg = []
            # bf16 casts of x1 up front so they land early in the DVE FIFO
            for g in range(NJ // AB):
                xb = srcp.tile([128, AB, D], BF, tag="src", name="xb")
                nc.vector.tensor_copy(xb[:], X[:, b, g * AB:(g + 1) * AB, :])
                xbg.append(xb)
            Z2h = z2p.tile([128, NJ, D], BF, tag="z2", name=f"z2_{b}")
            for l in range(L):
                fast = lnf_uniform[l] is not None
                # pass 1 (all groups): transposes, matmuls, PSUM-freeing
                # copies, ssq. No Silus/Exps in the ACT FIFO yet, so the
                # PE runs the whole layer without PSUM-release stalls.
                y1gs = []
                for g in range(NJ // AB):
                    jc0 = g * AB
                    src = xbg[g] if l == 0 else y2g[g]
                    if l == 0 and y2g[g] is None:
                        y2g[g] = srcp.tile([128, AB, D], BF, tag="src",
                                           name="y2")
                    y1g = y1p.tile([128, AB, FW], BF, tag="y1")
                    y1gs.append(y1g)
                    # two transpose waves of 2 chunks each
                    for w in range(AB // 2):
                        zdt = dtp.tile([128, 2 * DC, 128], BF, tag="dt",
                                       name="zdt")
                        nc.sync.dma_start_transpose(
                            zdt[:], src[:, w * 2:w * 2 + 2, :].rearrange(
                                "p a d -> p (a d)"))
                        for wi in range(2):
                            ti = w * 2 + wi
                            t = b * NJ + jc0 + ti
                            pff = ps_mm.tile([128, 1024], F32, tag="mm")
                            for c in range(DC):
                                nc.tensor.matmul(pff[:, 0:512],
                                                 zdt[:, wi * DC + c, :],
                                                 weights["wf"][:, l, c, 0:512],
                                                 start=(c == 0),
                                                 stop=(c == DC - 1))
                                nc.tensor.matmul(pff[:, 512:FW],
                                                 zdt[:, wi * DC + c, :],
                                                 weights["wf"][:, l, c, 512:FW],
                                                 start=(c == 0),
                                                 stop=(c == DC - 1))
                            if bf_nonzero[l]:
                                nc.vector.tensor_tensor(pff[:, 0:D],
                                                        pff[:, 0:D],
                                                        bft[l][:], op=Alu.add)
                            # copy 769 cols: dense output + its row-sum
                            nc.scalar.copy(y1g[:, ti, 0:D + 1],
                                           pff[:, 0:D + 1])
                            scr = scrp.tile([128, D], BF, tag="scr")
                            nc.vector.scalar_tensor_tensor(
                                scr[:], y1g[:, ti, 0:D], 0.0,
                                y1g[:, ti, 0:D],
                                op0=Alu.add, op1=Alu.mult,
                                accum_out=ssqC[:, l, t:t + 1])
                # pass 2 (all groups): stats epilogue, Silus, and (for the
                # second layer) the tail's add/abs + optional half-tail
                for g in range(NJ // AB):
                    jc0 = g * AB
                    y1g = y1gs[g]
                    t0 = b * NJ + jc0
                    ts = slice(t0, t0 + AB)
                    nc.vector.tensor_scalar(muC[:, l, ts], y1g[:, :, D],
                                            inv_d, None, op0=Alu.mult)
                    c_epi(b, l, g, jc0, 0, AB, fast, y1g, [None] * AB, y2g,
                          Z2h)
                    if dbg is not None and b == 0 and l == 0 and g == 0:
                        nc.gpsimd.dma_start(dbg["dbg_y2"], y2g[g][:])
                    if l == 1:
                        for ti in range(AB):
                            jc = jc0 + ti
                            xt = X[:, b, jc, :]
                            nc.vector.tensor_tensor(xt, xt, Z2h[:, jc, :],
                                                    op=Alu.add)
                            nc.vector.scalar_tensor_tensor(xt, xt, -1.0, xt,
                                                           op0=Alu.mult,
                                                           op1=Alu.max)
                        if tail_cb is not None:
                            tail_cb(g, Z2h)
            return Z2h

        def tail_part(b, Z2h, jlo, jn, fcell):
            # log_cosh(w) = |w| + log1p(exp(-2|w|)) - ln2; X already holds
            # |w| (add+abs ran inside phase C)
            # fence: depends on the covered layer-2 Silus, used as the Exp
            # bias so tail Exps can't interleave with Silus (table thrash)
            fscr = stat.tile([128, NJ], F32, tag="fscr", name="fscr")[:, 0:jn]
            nc.vector.tensor_scalar(fscr, Z2h[:, jlo:jlo + jn, 0], 0.0, 0.0,
                                    op0=Alu.mult, op1=Alu.mult,
                                    accum_out=fence[:, fcell:fcell + 1])
            for jc in range(jlo, jlo + jn):
                nc.scalar.activation(Z2h[:, jc, :], X[:, b, jc, :], Act.Exp,
                                     bias=fence[:, fcell:fcell + 1],
                                     scale=-2.0)
            for jc in range(jlo, jlo + jn):
                nc.scalar.activation(Z2h[:, jc, :], Z2h[:, jc, :], Act.Ln,
                                     bias=onet[:], scale=1.0)
            for half in range(jn // 2):
                j0 = jlo + half * 2
                for jc in range(j0, j0 + 2):
                    nc.vector.scalar_tensor_tensor(
                        X[:, b, jc, :], Z2h[:, jc, :], -LN2, X[:, b, jc, :],
                        op0=Alu.add, op1=Alu.add)
                nc.sync.dma_start(
                    out_d[b, j0 * 128:(j0 + 2) * 128, :].rearrange(
                        "(c p) d -> p c d", p=128),
                    X[:, b, j0:j0 + 2, :])

        phase_b(0)
        if dbg is not None:
            nc.gpsimd.dma_start(dbg["dbg_x1"][:, 0], X[:, 0])
        Z2_0 = phase_c(0)
        if dbg is not None:
            nc.gpsimd.dma_start(dbg["dbg_z2"], Z2_0[:])
        phase_b(1)
        if dbg is not None:
            nc.gpsimd.dma_start(dbg["dbg_x1"][:, 1], X[:, 1])
        tail_part(0, Z2_0, 0, NJ, 0)
        # batch 1's tail is split per 4-chunk half and emitted inside
        # phase C so the first half overlaps the final matmuls
        Z2_1 = phase_c(1, tail_cb=lambda g, Z: tail_part(1, Z, g * AB, AB,
                                                         2 + g))


def _prep(inputs):
    x = np.asarray(inputs["x"], np.float32)
    ln1_s = np.asarray(inputs["ln1_scale"], np.float32)
    ln1_b = np.asarray(inputs["ln1_bias"], np.float32)
    Wv = np.asarray(inputs["Wv"], np.float32)
    alpha = np.asarray(inputs["alpha"], np.float32)
    Wf = np.asarray(inputs["Wf"], np.float32)
    bfv = np.asarray(inputs["bf"], np.float32)
    lnf_s = np.asarray(inputs["lnf_scale"], np.float32)
    lnf_b = np.asarray(inputs["lnf_bias"], np.float32)

    Wv_flat = Wv.transpose(1, 0, 2).reshape(D, H * HS)
    Wvp = (ln1_s[:, None] * Wv_flat).astype(BF16)
    cv = (ln1_b @ Wv_flat).astype(np.float32)

    # Wf extended with a colsum column (row-sums of the dense output come
    # from the matmul itself) and zero padding to FW
    Wf_ext = np.zeros((L, D, FW), np.float32)
    Wf_ext[:, :, 0:D] = Wf
    Wf_ext[:, :, D] = Wf.sum(axis=2)

    ar = alpha[:, (-np.arange(N)) % N]
    ar2 = np.concatenate([ar, ar], axis=1)
    m_ = np.arange(NJ)[:, None, None]
    p_ = np.arange(128)[None, :, None]
    f_ = np.arange(128)[None, None, :]
    T = ar2[:, N + 128 * m_ + p_ - f_]                  # [H, NJ, 128, 128]
    # [128, H, NJ, 128]: partition-major so the DMA is contiguous
    tbank = np.ascontiguousarray(T.transpose(2, 0, 1, 3)).astype(BF16)

    cv_nonzero = bool(np.any(cv))
    bf_nonzero = tuple(bool(np.any(bfv[l])) for l in range(L))
    lnf_uniform = []
    for l in range(L):
        s, bb = lnf_s[l], lnf_b[l]
        if np.all(s == s[0]) and np.all(bb == bb[0]):
            lnf_uniform.append((float(s[0]), float(bb[0])))
        else:
            lnf_uniform.append(None)
    key = (cv_nonzero, bf_nonzero, tuple(lnf_uniform))

    # partition-major weight layouts for contiguous DMA:
    # wv: [D, H*HS] -> [128, DC, D_out];  wf: [L, D, FW] -> [128, L, DC, FW]
    wv_pm = np.ascontiguousarray(
        Wvp.reshape(DC, 128, D).transpose(1, 0, 2))
    wf_pm = np.ascontiguousarray(
        Wf_ext.astype(BF16).reshape(L, DC, 128, FW).transpose(2, 0, 1, 3))
    common = {
        "wv": wv_pm,
        "wf": wf_pm,
        "tbank": tbank,
        "cv": cv,
        "bfb": bfv,
        "lnfs": lnf_s,
        "lnfb": lnf_b,
    }
    return x, key, common, (cv_nonzero, bf_nonzero, lnf_uniform)


def kernel(**inputs):
    x, key, common, flags = _prep(inputs)
    if key not in _cache:
        _cache[key] = _build(*flags)
    nc = _cache[key]
    in_maps = []
    for i in range(NCORES):
        m = dict(common)
        m["xs"] = np.ascontiguousarray(x[i * BPC:(i + 1) * BPC])
        in_maps.append(m)
    res = run_bass_kernel_spmd(nc, in_maps, core_ids=list(range(NCORES)),
                               trace=TRACE, **TRACE_KW)
    kernel.last_result = res
    out = np.empty((B, N, D), np.float32)
    for i in range(NCORES):
        out[i * BPC:(i + 1) * BPC] = res.results[i]["out"]
    return out


# revision 52
# speedup vs baseline: 1.0455x; 1.0455x over previous
"""Trainium2 Bass kernel for nn_CoreBlock (circulant attention + 2-layer FFN).

Contract: kernel(**inputs) takes FULL unsharded inputs (as produced by
setup_inputs) and returns the FULL [16, 1024, 768] f32 output.

Strategy: pure data-parallel over batch - 8 NeuronCores x 2 batches each.
All weights replicated. Per core:
  phase A: LayerNorm(x) -> u (gpsimd), PE-transpose u, v = u_dt.T @ Wv' per
           token-chunk; results land in a resident V tensor in SBUF.
  phase B: per head h: circulant matmul using an 8-tile Toeplitz bank
           T[h,m]; both batches fused into one 128-wide moving operand and
           multi-chunk (<=512 free) moving slices, so the whole head is
           ~22 large matmuls instead of 128 tiny ones. Residual-added into
           X (X becomes x1 = x + y).
  phase C: 2x [Dense -> LayerNorm -> swish], software-pipelined in groups
           of 4 token chunks. rstd comes from a DVE Newton rsqrt (bit-hack
           seed + 2 iterations) so the scalar engine never loads the Sqrt
           table and the Silu table stays resident for the whole phase.
  tail:    log_cosh(w) = softplus(2w) - w - ln2 (one ACT pass, no abs),
           group-batched output DMA.

Matmul operands are bf16 (full-rate PE, fp32 PSUM accumulation); stats and
elementwise math fp32. PSUM->SBUF copies are split between the scalar and
vector engines to balance them; gpsimd takes pure-SBUF elementwise work.
"""

import math
import numpy as np
import ml_dtypes

import concourse.bass as bass
import concourse.tile as tile
from concourse import bacc, mybir
from concourse.bass_utils import run_bass_kernel_spmd

BF16 = ml_dtypes.bfloat16

B, N, D = 16, 1024, 768
H, HS, L = 12, 64, 2
EPS = 1e-6
NCORES = 8
BPC = B // NCORES          # batches per core
NJ = N // 128              # token chunks per batch (8)
NT = BPC * NJ              # token chunks per core (16)
DC = D // 128              # feature chunks (6)
AB = 4                     # chunks per pipeline group

F32 = mybir.dt.float32
I32 = mybir.dt.int32
BF = mybir.dt.bfloat16
Alu = mybir.AluOpType
Act = mybir.ActivationFunctionType

LN2 = math.log(2.0)
# fp32 whose bit pattern is 0x5f3759df (fast-rsqrt magic constant)
MAGIC_F = float(np.int32(0x5F3759DF).view(np.float32))

USE_SOFTPLUS = False       # no Softplus table on TRN2; use Exp+Ln log_cosh
USE_GPSIMD = False         # gpsimd tensor ops measured ~20x slower than DVE

TRACE = False              # test harness sets this for profiling runs
TRACE_KW = {}

_cache = {}


class _Bacc(bacc.Bacc):
    """Bacc whose activation-table chooser sees Exp and Ln only in the
    combined natural_log_exp_and_others set, so the log_cosh tail needs one
    table load instead of alternating exp/ln loads. List order (and thus
    act_func_set_id) is unchanged; only the per-set membership used for
    choosing is filtered."""

    def insert_act_table_loads(self):
        has_activation = any(
            isinstance(i, mybir.InstActivation)
            for b in self.main_func.blocks
            for i in b.instructions
        )
        if not has_activation:
            return
        from concourse.hw_specs import get_activation_tables
        Fn = mybir.ActivationFunctionType
        tables = []
        for name, fns in get_activation_tables(self.m.arch).items():
            if name != "natural_log_exp_and_others":
                fns = fns - {Fn.Exp, Fn.Ln}
            tables.append((name, fns))
        import concourse._compat as _compat  # noqa: F401
        from concourse.bacc import _bass_rust
        _bass_rust.insert_act_table_loads(self, tables)


def _build(cv_nonzero, bf_nonzero, lnf_uniform):
    nc = _Bacc("TRN2", target_bir_lowering=False, debug=False)

    xs = nc.dram_tensor("xs", (BPC, N, D), F32, kind="ExternalInput").ap()
    wv = nc.dram_tensor("wv", (D, D), BF, kind="ExternalInput").ap()
    wf = nc.dram_tensor("wf", (L, D, D), BF, kind="ExternalInput").ap()
    tb_d = nc.dram_tensor("tbank", (H, 128, NJ * 128), BF, kind="ExternalInput").ap()
    id32 = nc.dram_tensor("id32", (128, 128), F32, kind="ExternalInput").ap()
    idbf = nc.dram_tensor("idbf", (128, 128), BF, kind="ExternalInput").ap()
    cv_d = nc.dram_tensor("cv", (D,), F32, kind="ExternalInput").ap()
    bf_d = nc.dram_tensor("bfb", (L, D), F32, kind="ExternalInput").ap()
    lnfs_d = nc.dram_tensor("lnfs", (L, D), F32, kind="ExternalInput").ap()
    lnfb_d = nc.dram_tensor("lnfb", (L, D), F32, kind="ExternalInput").ap()
    out_d = nc.dram_tensor("out", (BPC, N, D), F32, kind="ExternalOutput").ap()

    with tile.TileContext(nc) as tc:
        _emit(nc, tc, xs, wv, wf, tb_d, id32, idbf, cv_d, bf_d, lnfs_d, lnfb_d,
              out_d, cv_nonzero, bf_nonzero, lnf_uniform)
    nc.compile()
    return nc


def _newton_rsqrt(nc, pool, dst, var_ap, magict, g):
    """dst[128, g] = 1/sqrt(var + EPS) via bit-hack seed + 2 Newton steps.
    All on the vector engine; no activation tables involved."""
    vv = pool.tile([128, 16], F32, tag="nvv", name="nvv")[:, 0:g]
    nc.vector.tensor_scalar(vv, var_ap, EPS, None, op0=Alu.add)
    y0 = pool.tile([128, 16], F32, tag="ny0", name="ny0")[:, 0:g]
    nc.vector.tensor_scalar(y0.bitcast(I32), vv.bitcast(I32), 1, None,
                            op0=Alu.logical_shift_right)
    nc.vector.tensor_tensor(y0.bitcast(I32), magict[:, 0:g].bitcast(I32),
                            y0.bitcast(I32), op=Alu.subtract)
    t1 = pool.tile([128, 16], F32, tag="nt1", name="nt1")[:, 0:g]
    for it in range(2):
        nc.vector.tensor_tensor(t1, y0, y0, op=Alu.mult)
        nc.vector.tensor_tensor(t1, t1, vv, op=Alu.mult)
        nc.vector.tensor_scalar(t1, t1, -0.5, 1.5, op0=Alu.mult, op1=Alu.add)
        nc.vector.tensor_tensor(dst if it == 1 else y0, y0, t1, op=Alu.mult)


def _emit(nc, tc, xs, wv, wf, tb_d, id32, idbf, cv_d, bf_d, lnfs_d, lnfb_d,
          out_d, cv_nonzero, bf_nonzero, lnf_uniform):
    from contextlib import ExitStack
    gps = nc.gpsimd if USE_GPSIMD else nc.vector
    ctx = ExitStack()
    with ctx:
        consts = ctx.enter_context(tc.tile_pool(name="consts", bufs=1))
        xpool = ctx.enter_context(tc.tile_pool(name="xpool", bufs=1))
        vpool = ctx.enter_context(tc.tile_pool(name="vpool", bufs=1))
        acts = ctx.enter_context(tc.tile_pool(name="acts", bufs=18))
        upool = ctx.enter_context(tc.tile_pool(name="upool", bufs=3))
        x1p = ctx.enter_context(tc.tile_pool(name="x1p", bufs=3))
        dtp = ctx.enter_context(tc.tile_pool(name="dtp", bufs=3))
        stat = ctx.enter_context(tc.tile_pool(name="stat", bufs=4))
        statp = ctx.enter_context(tc.tile_pool(name="statp", bufs=2))
        scrp = ctx.enter_context(tc.tile_pool(name="scrp", bufs=2))
        wkp = ctx.enter_context(tc.tile_pool(name="wkp", bufs=2))
        ps_tr = ctx.enter_context(tc.tile_pool(name="ps_tr", bufs=2, space="PSUM"))
        ps_mm = ctx.enter_context(tc.tile_pool(name="ps_mm", bufs=3, space="PSUM"))

        # ---- constants ----
        wv_s = consts.tile([128, DC, D], BF, tag="wv")
        wf_s = consts.tile([128, L, DC, D], BF, tag="wf")
        tb_s = consts.tile([128, H, NJ, 128], BF, tag="tb")
        magict = consts.tile([128, 16], F32, tag="magic")
        nc.vector.memset(magict[:], MAGIC_F)
        onet = consts.tile([128, 1], F32, tag="one")
        nc.vector.memset(onet[:], 1.0)
        zerot = consts.tile([128, 1], F32, tag="zero")
        nc.vector.memset(zerot[:], 0.0)

        # weight/constant DMAs go through the gpsimd software-DGE queue (a
        # third parallel DMA path); wv is split per 128-block so the first
        # matmuls can start before the whole tensor lands.
        ibf = consts.tile([128, 128], BF, tag="ibf")
        nc.gpsimd.dma_start(ibf[:], idbf)
        nc.gpsimd.dma_start(wv_s[:], wv.rearrange("(c p) f -> p c f", p=128))

        cvt = None
        if cv_nonzero:
            cvt = consts.tile([128, D], F32, tag="cv")
            nc.gpsimd.dma_start(cvt[:], cv_d.to_broadcast((128, D)))
        bft = [None] * L
        lnfst = [None] * L
        lnfbt = [None] * L
        for l in range(L):
            if bf_nonzero[l]:
                bft[l] = consts.tile([128, D], F32, tag=f"bf{l}")
                nc.gpsimd.dma_start(bft[l][:], bf_d[l].to_broadcast((128, D)))
            if lnf_uniform[l] is None:
                lnfst[l] = consts.tile([128, D], F32, tag=f"lnfs{l}")
                nc.gpsimd.dma_start(lnfst[l][:], lnfs_d[l].to_broadcast((128, D)))
                lnfbt[l] = consts.tile([128, D], F32, tag=f"lnfb{l}")
                nc.gpsimd.dma_start(lnfbt[l][:], lnfb_d[l].to_broadcast((128, D)))

        # ---- resident tensors ----
        X = xpool.tile([128, BPC, NJ, D], F32, tag="X")         # x, then x1
        V = vpool.tile([128, H, NJ, BPC, HS], BF, tag="V")      # per-head values

        # ================= phase A: LN + v-projection =================
        mvA = statp.tile([128, NT, 2], F32, tag="mvA")
        rsA = statp.tile([128, NT], F32, tag="rsA")
        for t0 in range(0, NT, AB):
            for t in range(t0, t0 + AB):
                b, jc = divmod(t, NJ)
                eng = nc.sync if t % 2 == 0 else nc.scalar
                eng.dma_start(X[:, b, jc, :],
                              xs[b, jc * 128:(jc + 1) * 128, :])
            for t in range(t0, t0 + AB):
                b, jc = divmod(t, NJ)
                xt = X[:, b, jc, :]
                st = stat.tile([128, 2, 6], F32, tag="bst")
                nc.vector.bn_stats(st[:, 0, :], xt[:, 0:512])
                nc.vector.bn_stats(st[:, 1, :], xt[:, 512:D])
                nc.vector.bn_aggr(mvA[:, t, :], st[:])
            # batched newton per group: the per-chunk version cost ~13 tiny
            # DVE ops (~2us) per chunk in pure instruction overhead
            _newton_rsqrt(nc, stat, rsA[:, t0:t0 + AB],
                          mvA[:, t0:t0 + AB, 1], magict, AB)
            if t0 == 0:
                nc.gpsimd.dma_start(
                    tb_s[:], tb_d.rearrange("h p (m f) -> p h m f", m=NJ))
            if t0 == AB:
                nc.gpsimd.dma_start(
                    wf_s[:], wf.rearrange("l (c p) f -> p l c f", p=128))
            for t in range(t0, t0 + AB):
                b, jc = divmod(t, NJ)
                xt = X[:, b, jc, :]
                u = upool.tile([128, D], BF, tag="u")
                gps.tensor_scalar(u[:], xt, mvA[:, t, 0:1], rsA[:, t:t + 1],
                                  op0=Alu.subtract, op1=Alu.mult)
                # PE transposes here: phase A's XBARs would inherit DMA
                # semaphore-slot WAR guards on the in-flight x transfers
                # (~40us); the PE path starts as soon as u(0) is ready.
                ptr = ps_tr.tile([128, D], BF, tag="tr")
                for c in range(DC):
                    nc.tensor.transpose(ptr[:, c * 128:(c + 1) * 128],
                                        u[:, c * 128:(c + 1) * 128], ibf[:])
                udt = dtp.tile([128, D], BF, tag="udta", name="udt")
                nc.scalar.copy(udt[:], ptr[:])
                pv = ps_mm.tile([128, NJ, 128], F32, tag="mm")
                pvf = pv[:].rearrange("p a b -> p (a b)")
                for c in range(DC):
                    nc.tensor.matmul(pvf[:, 0:512],
                                     udt[:, c * 128:(c + 1) * 128],
                                     wv_s[:, c, 0:512],
                                     start=(c == 0), stop=(c == DC - 1))
                    nc.tensor.matmul(pvf[:, 512:D],
                                     udt[:, c * 128:(c + 1) * 128],
                                     wv_s[:, c, 512:D],
                                     start=(c == 0), stop=(c == DC - 1))
                pv3 = pvf[:, 0:D].rearrange("p (h k) -> p h k", h=H)
                if cv_nonzero:
                    cv3 = cvt[:].rearrange("p (h k) -> p h k", h=H)
                    nc.vector.tensor_tensor(V[:, :, jc, b, :], pv3, cv3,
                                            op=Alu.add)
                else:
                    # DVE is phase A's critical engine; the cast goes to ACT
                    nc.scalar.copy(V[:, :, jc, b, :], pv3)

        # ================= phase B: circulant attention =================
        # y[ic] = sum_m T[m] @ V[(ic+m) % NJ], both batches fused in the
        # moving operand (free = jc-run * BPC*HS, up to 512).
        for h in range(H):
            pc = ps_mm.tile([128, NJ, BPC * HS], F32, tag="mm")
            for m in range(NJ):
                for ic0, jc0, ln in ((0, m, NJ - m), (NJ - m, 0, m)):
                    p0 = 0
                    while p0 < ln:
                        pl = min(4, ln - p0)
                        nc.tensor.matmul(
                            pc[:, ic0 + p0:ic0 + p0 + pl, :],
                            tb_s[:, h, m, :],
                            V[:, h, jc0 + p0:jc0 + p0 + pl, :, :],
                            start=(m == 0), stop=(m == NJ - 1),
                            skip_group_check=True)
                        p0 += pl
            for b in range(BPC):
                xap = X[:, b, :, h * HS:(h + 1) * HS]
                nc.vector.tensor_tensor(xap, xap, pc[:, :, b * HS:(b + 1) * HS],
                                        op=Alu.add)

        # ================= phase C + tail, in two half-batches =================
        # Each half (8 chunks = one batch) runs L1 -> L2 -> log_cosh tail;
        # the second half's matmuls overlap the first half's scalar/vector
        # tail so the PE never sits idle for long.
        inv_d = 1.0 / D
        zcur = [None] * NT
        # layer-2 outputs land in one resident tensor (reusing the Toeplitz
        # bank's SBUF slot, dead after phase B) so the tail can fence on
        # a whole half at once; the bf16 exp buffer reuses V's slot.
        Z2 = consts.tile([128, NT, D], BF, tag="tb", name="Z2")
        awl = vpool.tile([128, NT, D], BF, tag="V")
        stats_t = {}
        for l in range(L):
            stats_t[l] = dict(
                sums=statp.tile([128, NT], F32, tag=f"sum{l}", name="sums"),
                ssq=statp.tile([128, NT], F32, tag=f"ssq{l}", name="ssq"),
                muA=statp.tile([128, NT], F32, tag=f"mu{l}", name="muA"),
                rsF=statp.tile([128, NT], F32, tag=f"rs{l}", name="rsF"),
                biasF=statp.tile([128, NT], F32, tag=f"bi{l}", name="biasF"),
            )
        fence = statp.tile([128, NT // AB], F32, tag="fence")

        for l in range(L):
                fast = lnf_uniform[l] is not None
                stt = stats_t[l]
                sums, ssq = stt["sums"], stt["ssq"]
                muA, rsF, biasF = stt["muA"], stt["rsF"], stt["biasF"]
                for g0 in range(0, NT, AB):
                    for t in range(g0, g0 + AB):
                        b, jc = divmod(t, NJ)
                        if l == 0:
                            # bf16 copy of x1 feeds the 2-byte XBAR transpose
                            src = x1p.tile([128, D], BF, tag="x1b", name="x1b")
                            nc.vector.tensor_copy(src[:], X[:, b, jc, :])
                            src = src[:]
                        else:
                            src = zcur[t][:]
                        zdt = dtp.tile([128, DC, 128], BF, tag="udt")
                        nc.sync.dma_start_transpose(zdt[:], src)
                        pf = ps_mm.tile([128, NJ, 128], F32, tag="mm")
                        pff = pf[:].rearrange("p a b -> p (a b)")
                        for c in range(DC):
                            nc.tensor.matmul(pff[:, 0:512],
                                             zdt[:, c, :],
                                             wf_s[:, l, c, 0:512],
                                             start=(c == 0), stop=(c == DC - 1))
                            nc.tensor.matmul(pff[:, 512:D],
                                             zdt[:, c, :],
                                             wf_s[:, l, c, 512:D],
                                             start=(c == 0), stop=(c == DC - 1))
                        if bf_nonzero[l]:
                            nc.vector.tensor_tensor(pff[:, 0:D], pff[:, 0:D],
                                                    bft[l][:], op=Alu.add)
                        y = acts.tile([128, D], BF, tag="acts")
                        nc.scalar.activation(y[:], pff[:, 0:D], Act.Copy,
                                             accum_out=sums[:, t:t + 1])
                        scr = scrp.tile([128, D], BF, tag="scr")
                        nc.vector.scalar_tensor_tensor(
                            scr[:], y[:], 0.0, y[:], op0=Alu.add, op1=Alu.mult,
                            accum_out=ssq[:, t:t + 1])
                        zcur[t] = y
                    # group epilogue: var -> rstd (DVE Newton) -> Silu
                    g = slice(g0, g0 + AB)
                    nc.vector.tensor_scalar(muA[:, g], sums[:, g], inv_d, None,
                                            op0=Alu.mult)
                    m2 = stat.tile([128, 16], F32, tag="m2", name="m2")[:, 0:AB]
                    nc.vector.tensor_scalar(m2, ssq[:, g], inv_d, None,
                                            op0=Alu.mult)
                    var = stat.tile([128, 16], F32, tag="var", name="var")[:, 0:AB]
                    nc.vector.scalar_tensor_tensor(var, muA[:, g], -1.0,
                                                   muA[:, g], op0=Alu.mult,
                                                   op1=Alu.mult)
                    nc.vector.tensor_tensor(var, m2, var, op=Alu.add)
                    _newton_rsqrt(nc, stat, rsF[:, g], var, magict, AB)
                    if fast:
                        cs, cb = lnf_uniform[l]
                        if cs != 1.0:
                            nc.vector.tensor_scalar(rsF[:, g], rsF[:, g],
                                                    float(cs), None,
                                                    op0=Alu.mult)
                        nc.vector.scalar_tensor_tensor(biasF[:, g], muA[:, g],
                                                       -1.0, rsF[:, g],
                                                       op0=Alu.mult,
                                                       op1=Alu.mult)
                        if cb != 0.0:
                            nc.vector.tensor_scalar(biasF[:, g], biasF[:, g],
                                                    float(cb), None,
                                                    op0=Alu.add)
                        for t in range(g0, g0 + AB):
                            y = zcur[t]
                            dst = y[:] if l == 0 else Z2[:, t, :]
                            nc.scalar.activation(dst, y[:], Act.Silu,
                                                 bias=biasF[:, t:t + 1],
                                                 scale=rsF[:, t:t + 1])
                            if l == 1:
                                zcur[t] = None
                    else:
                        for t in range(g0, g0 + AB):
                            y = zcur[t]
                            tmp = acts.tile([128, D], BF, tag="acts")
                            nc.vector.tensor_scalar(tmp[:], y[:],
                                                    muA[:, t:t + 1],
                                                    rsF[:, t:t + 1],
                                                    op0=Alu.subtract,
                                                    op1=Alu.mult)
                            nc.vector.tensor_tensor(tmp[:], tmp[:], lnfst[l][:],
                                                    op=Alu.mult)
                            dst = tmp[:] if l == 0 else Z2[:, t, :]
                            nc.vector.tensor_tensor(dst, tmp[:], lnfbt[l][:],
                                                    op=Alu.add)
                            nc.scalar.activation(dst, dst, Act.Silu,
                                                 bias=zerot[:])
                            zcur[t] = tmp if l == 0 else None

        # ---- tail, in two halves: log_cosh(w) = |w| + log1p(exp(-2|w|)) - ln2
        # Half 1's exp/ln overlaps layer 2's back half; per-chunk output
        # DMAs overlap the store transfers with the remaining compute.
        for h0 in range(0, NT, NJ):
            hh = h0 // NJ
            for t in range(h0, h0 + NJ):
                b, jc = divmod(t, NJ)
                xt = X[:, b, jc, :]
                nc.vector.tensor_tensor(xt, xt, Z2[:, t, :], op=Alu.add)
                nc.vector.scalar_tensor_tensor(xt, xt, -1.0, xt,
                                               op0=Alu.mult, op1=Alu.max)
            # fence: a zero [128,1] that depends on every layer-2 Silu of
            # this half; used as the Exp bias so the scheduler cannot
            # interleave tail Exps between Silus (activation-table thrash).
            fscr = stat.tile([128, NJ], F32, tag="fscr", name="fscr")
            nc.vector.tensor_scalar(fscr[:], Z2[:, h0:h0 + NJ, 0], 0.0, 0.0,
                                    op0=Alu.mult, op1=Alu.mult,
                                    accum_out=fence[:, hh:hh + 1])
            for t in range(h0, h0 + NJ):
                b, jc = divmod(t, NJ)
                nc.scalar.activation(awl[:, t, :], X[:, b, jc, :], Act.Exp,
                                     bias=fence[:, hh:hh + 1], scale=-2.0)
            for t in range(h0, h0 + NJ):
                nc.scalar.activation(awl[:, t, :], awl[:, t, :], Act.Ln,
                                     bias=onet[:], scale=1.0)
            for t in range(h0, h0 + NJ):
                b, jc = divmod(t, NJ)
                sp = wkp.tile([128, D], F32, tag="sp", name="sp", bufs=4)
                nc.vector.scalar_tensor_tensor(sp[:], awl[:, t, :],
                                               -LN2, X[:, b, jc, :],
                                               op0=Alu.add, op1=Alu.add)
                nc.scalar.dma_start(out_d[b, jc * 128:(jc + 1) * 128, :], sp[:])


def _prep(inputs):
    x = np.asarray(inputs["x"], np.float32)
    ln1_s = np.asarray(inputs["ln1_scale"], np.float32)
    ln1_b = np.asarray(inputs["ln1_bias"], np.float32)
    Wv = np.asarray(inputs["Wv"], np.float32)
    alpha = np.asarray(inputs["alpha"], np.float32)
    Wf = np.asarray(inputs["Wf"], np.float32)
    bfv = np.asarray(inputs["bf"], np.float32)
    lnf_s = np.asarray(inputs["lnf_scale"], np.float32)
    lnf_b = np.asarray(inputs["lnf_bias"], np.float32)

    Wv_flat = Wv.transpose(1, 0, 2).reshape(D, H * HS)
    Wvp = (ln1_s[:, None] * Wv_flat).astype(BF16)
    cv = (ln1_b @ Wv_flat).astype(np.float32)

    ar = alpha[:, (-np.arange(N)) % N]
    ar2 = np.concatenate([ar, ar], axis=1)
    m_ = np.arange(NJ)[:, None, None]
    p_ = np.arange(128)[None, :, None]
    f_ = np.arange(128)[None, None, :]
    T = ar2[:, N + 128 * m_ + p_ - f_]                  # [H, NJ, 128, 128]
    tbank = np.ascontiguousarray(
        T.transpose(0, 2, 1, 3).reshape(H, 128, NJ * 128)).astype(BF16)

    cv_nonzero = bool(np.any(cv))
    bf_nonzero = tuple(bool(np.any(bfv[l])) for l in range(L))
    lnf_uniform = []
    for l in range(L):
        s, bb = lnf_s[l], lnf_b[l]
        if np.all(s == s[0]) and np.all(bb == bb[0]):
            lnf_uniform.append((float(s[0]), float(bb[0])))
        else:
            lnf_uniform.append(None)
    key = (cv_nonzero, bf_nonzero, tuple(lnf_uniform))

    common = {
        "wv": np.ascontiguousarray(Wvp),
        "wf": Wf.astype(BF16),
        "tbank": tbank,
        "id32": np.eye(128, dtype=np.float32),
        "idbf": np.eye(128, dtype=BF16),
        "cv": cv,
        "bfb": bfv,
        "lnfs": lnf_s,
        "lnfb": lnf_b,
    }
    return x, key, common, (cv_nonzero, bf_nonzero, lnf_uniform)


def kernel(**inputs):
    x, key, common, flags = _prep(inputs)
    if key not in _cache:
        _cache[key] = _build(*flags)
    nc = _cache[key]
    in_maps = []
    for i in range(NCORES):
        m = dict(common)
        m["xs"] = np.ascontiguousarray(x[i * BPC:(i + 1) * BPC])
        in_maps.append(m)
    res = run_bass_kernel_spmd(nc, in_maps, core_ids=list(range(NCORES)),
                               trace=TRACE, **TRACE_KW)
    kernel.last_result = res
    out = np.empty((B, N, D), np.float32)
    for i in range(NCORES):
        out[i * BPC:(i + 1) * BPC] = res.results[i]["out"]
    return out

